# revision 10
# baseline (speedup 1.0000x reference)
"""GAttNHP model as a Trainium2 Bass/Tile kernel.

Data-parallel over batch: 16 batches -> 8 cores x 2 batches.  All matmuls
bf16 (f32 PSUM accumulation).  Feature-major layouts throughout; linear
biases folded in via augmented ones-rows; attention softmax denominators
via an appended ones-column on V.  The intensity head streams int_w in 16
column tiles of 500 entities, double-buffered against PE.
"""

import sys
import types

import numpy as np
import ml_dtypes

# ---------------------------------------------------------------------------
# environment shims (missing antenv.axon_hooks on this image; no bucket access)
# ---------------------------------------------------------------------------
import antenv  # noqa: F401

if "antenv.axon_hooks" not in sys.modules:
    _m = types.ModuleType("antenv.axon_hooks")
    _m._hook = None
    _m.set_axon_ntff_profile_hook = lambda h: setattr(_m, "_hook", h)
    _m.get_axon_ntff_profile_hook = lambda: _m._hook
    sys.modules["antenv.axon_hooks"] = _m
    try:
        from trn_agent_boot.trn_boot import _ntff_profile_via_ctypes

        _m.set_axon_ntff_profile_hook(
            _ntff_profile_via_ctypes("/opt/axon/libaxon_pjrt.so")
        )
    except Exception:
        pass

import concourse.bass as bass
import concourse.mybir as mybir
import concourse.tile as tile
import concourse.bass_utils as bass_utils
from concourse.bass import IndirectOffsetOnAxis
from concourse.bass_utils import run_bass_kernel_spmd
from concourse.masks import make_identity
from concourse.vector_clock import ScopedClock as _ScopedClock

bass_utils.upload_artifacts = lambda tmpdir: tmpdir


def _drain_and_barrier_split(self, tick_clock, wait_clock):
    # stock tail drain carries >2 sem waits on one instruction, which this
    # walrus build rejects; split into one wait per SP nop.
    nc = self.nc
    probe = nc.sync.nop()
    wait_clock.add_sem_waits(probe.ins, _ScopedClock({None: tick_clock.global_clock}))
    waits = list(probe.ins.sync_info.on_wait) if probe.ins.sync_info else []
    if len(waits) > 1:
        probe.ins.sync_info = mybir.SyncInfo(on_update=[], on_wait=waits[:1])
        for w in waits[1:]:
            n = nc.sync.nop()
            n.ins.sync_info = mybir.SyncInfo(on_update=[], on_wait=[w])
    nc.sync.drain()
    nc.all_engine_barrier()
    popped = nc._tile_sem_poison_stack.pop()
    assert popped is self._sem_poison
    nc.clear_and_free_semaphores(list(self.sems.allocated().values()))
    nc.all_engine_barrier()


tile.TileContext._drain_and_barrier = _drain_and_barrier_split

MAX_WAITS = 1


def _split_excess_waits(nc):
    # this walrus build rejects instructions carrying more than ~2 sem waits;
    # move excess waits onto same-engine NOPs spliced in just before.
    for f in nc.m.functions:
        for blk in f.blocks:
            insts = blk.instructions
            out = []
            changed = False
            for inst in insts:
                si = inst.sync_info
                if si is not None and len(si.on_wait) > MAX_WAITS:
                    waits = list(si.on_wait)
                    extra = waits[: len(waits) - MAX_WAITS]
                    keep = waits[len(waits) - MAX_WAITS :]
                    for j in range(0, len(extra), MAX_WAITS):
                        nop = mybir.InstNoOp(
                            name=f"{inst.name}-w{j}", ins=[], outs=[])
                        nop.engine = inst.engine
                        nop.sync_info = mybir.SyncInfo(
                            on_update=[], on_wait=extra[j : j + MAX_WAITS])
                        out.append(nop)
                    inst.sync_info = mybir.SyncInfo(
                        on_update=list(si.on_update), on_wait=keep)
                    changed = True
                out.append(inst)
            if changed:
                blk.instructions = out

# ---------------------------------------------------------------------------
# model constants
# ---------------------------------------------------------------------------
NE = 8000      # n_entity
NR = 100       # n_rel
NG = 64        # n_groups
HID = 256
DM = 256       # d_model
NL = 2         # layers
NH = 4         # heads
HD = 64        # head dim
GP = 64        # group proj dim
DT = DM * NL   # 512
DF = DT + 2 * HID  # 1024
B, Lfull = 16, 512
LH = 511
L = 512        # padded seq
NCORE = 8
B2 = 2         # batches per core
QC = 4         # seq chunks of 128
DK = 9         # K chunks over DF+1 augmented (8 full + 1 partial)
NT = 16        # intensity column tiles
NTW = 500      # entities per tile
F32 = mybir.dt.float32
BF16 = mybir.dt.bfloat16
I32 = mybir.dt.int32
AF = mybir.ActivationFunctionType
AX = mybir.AxisListType
OP = mybir.AluOpType
PI_2 = float(np.pi / 2)

_cached = None


def _build():
    nc = bass.Bass()
    dt_in = {}

    def din(name, shape, dtype):
        dt_in[name] = nc.dram_tensor(name, shape, dtype, kind="ExternalInput")
        return dt_in[name]

    obj_emb = din("obj_emb", [NE, DM], BF16)
    sub_emb = din("sub_emb", [NE, HID], BF16)
    rel_emb = din("rel_emb", [NR, HID], BF16)
    gmap = din("gmap", [NE * NR, 1], I32)
    wq_d = din("wq", [128, NL, 2, DM], BF16)
    wk_d = din("wk", [128, NL, 2, DM], BF16)
    wv_d = din("wv", [128, NL, 2, DM], BF16)
    wo_d = din("wo", [128, NL, 2, DM], BF16)
    gpw_d = din("gp_w", [128, 8, GP], BF16)
    gaiw_d = din("ga_in_w", [GP, 3 * GP], BF16)
    gaow_d = din("ga_out_w", [GP, GP], BF16)
    ffn1_d = din("ffn_w1", [GP, GP], BF16)
    ffn2_d = din("ffn_w2", [GP, GP], BF16)
    mgw_d = din("mg_w", [128, DK, DF], BF16)
    intw_d = din("int_w", [128, DK, NE], BF16)
    cr_d = din("consts_row", [1, 768], F32)
    gab_d = din("gab_col", [3 * GP, 1], F32)
    fr_d = din("freqs", [1, 128], F32)
    objs_d = din("objs_h", [B2, L], I32)
    subs_d = din("subs_h", [B2, L], I32)
    marks_d = din("marks_h", [B2, L], I32)
    sid_d = din("sid", [B2, 1], I32)
    rid_d = din("rid", [B2, 1], I32)
    th_d = din("th", [B2, L], F32)
    tq_d = din("tq", [B2, L], F32)
    dth_d = din("dth", [B2, L], F32)
    fm_d = din("fm", [B2, L], F32)
    out_d = nc.dram_tensor("out", [B2, L, NE], F32, kind="ExternalOutput")

    with tile.TileContext(nc) as tc:
        with (
            tc.tile_pool(name="cp", bufs=1) as cp,
            tc.tile_pool(name="wp", bufs=1) as wp,
            tc.tile_pool(name="rot", bufs=2) as rot,
            tc.tile_pool(name="intw", bufs=2) as iwp,
            tc.tile_pool(name="ost", bufs=3) as ost,
            tc.tile_pool(name="psA", bufs=5, space="PSUM") as psA,
            tc.tile_pool(name="psI", bufs=3, space="PSUM") as psI,
        ):
            # ---------------- constants ----------------
            ident_f = cp.tile([128, 128], F32, tag="identf")
            make_identity(nc, ident_f[:])
            ident_b = cp.tile([128, 128], BF16, tag="identb")
            make_identity(nc, ident_b[:])
            # causal 0/1 mask for the diagonal 128-block: 1 where j >= p
            tri_i = cp.tile([128, 128], I32, tag="trii")
            nc.gpsimd.iota(tri_i[:], pattern=[[1, 128]], base=0, channel_multiplier=-1)
            tri_f = cp.tile([128, 128], F32, tag="trif")
            nc.vector.tensor_copy(tri_f[:], tri_i[:])
            tri = cp.tile([128, 128], BF16, tag="tri")
            nc.vector.tensor_scalar(tri[:], tri_f[:], 0.0, None, op0=OP.is_ge)
            # iota over groups (same row 0..63 on every partition)
            iota_i = cp.tile([128, NG], I32, tag="iotai")
            nc.gpsimd.iota(iota_i[:], pattern=[[1, NG]], base=0, channel_multiplier=0)
            iota_g = cp.tile([128, NG], F32, tag="iotag")
            nc.vector.tensor_copy(iota_g[:], iota_i[:])
            ones_row = cp.tile([1, L], BF16, tag="onesrow")
            nc.vector.memset(ones_row[:], 1.0)
            ones1f = cp.tile([1, 64], F32, tag="ones1f")
            nc.vector.memset(ones1f[:], 1.0)
            pi2 = cp.tile([128, 1], F32, tag="pi2")
            nc.vector.memset(pi2[:], PI_2)
            selA = cp.tile([1, 128], F32, tag="selA")
            nc.vector.memset(selA[:], 0.0)
            nc.vector.memset(selA[0:1, 0:64], 1.0)
            selB = cp.tile([1, 128], F32, tag="selB")
            nc.vector.memset(selB[:], 0.0)
            nc.vector.memset(selB[0:1, 64:128], 1.0)

            # ---------------- weights to SBUF ----------------
            wq = wp.tile([128, NL, 2, DM], BF16, tag="wq")
            wk = wp.tile([128, NL, 2, DM], BF16, tag="wk")
            wv = wp.tile([128, NL, 2, DM], BF16, tag="wv")
            wo = wp.tile([128, NL, 2, DM], BF16, tag="wo")
            for t, d in ((wq, wq_d), (wk, wk_d), (wv, wv_d), (wo, wo_d)):
                nc.sync.dma_start(t[:], d[:])
            gpw = wp.tile([128, 8, GP], BF16, tag="gpw")
            nc.sync.dma_start(gpw[:], gpw_d[:])
            gaiw = wp.tile([GP, 3 * GP], BF16, tag="gaiw")
            nc.sync.dma_start(gaiw[:], gaiw_d[:])
            gaow = wp.tile([GP, GP], BF16, tag="gaow")
            nc.sync.dma_start(gaow[:], gaow_d[:])
            ffn1 = wp.tile([GP, GP], BF16, tag="ffn1")
            nc.sync.dma_start(ffn1[:], ffn1_d[:])
            ffn2 = wp.tile([GP, GP], BF16, tag="ffn2")
            nc.sync.dma_start(ffn2[:], ffn2_d[:])
            mgw = wp.tile([128, DK, DF], BF16, tag="mgw")
            nc.sync.dma_start(mgw[:], mgw_d[:])
            freqs = wp.tile([1, 128], F32, tag="freqs")
            nc.sync.dma_start(freqs[:], fr_d[:])
            cr_sb = wp.tile([1, 768], F32, tag="crsb")
            nc.sync.dma_start(cr_sb[:], cr_d[:])
            gab1 = wp.tile([128, 1], F32, tag="gab1")
            nc.sync.dma_start(gab1[:], gab_d[0:128, :])
            gab2 = wp.tile([64, 1], F32, tag="gab2")
            nc.sync.dma_start(gab2[:], gab_d[128:192, :])
            sid_sb = wp.tile([B2, 1], I32, tag="sidsb")
            nc.sync.dma_start(sid_sb[:], sid_d[:])
            rid_sb = wp.tile([B2, 1], I32, tag="ridsb")
            nc.sync.dma_start(rid_sb[:], rid_d[:])

            # broadcast small consts row -> [64, 768]
            consts = wp.tile([64, 768], F32, tag="consts")
            for s in range(0, 768, 512):
                e = min(s + 512, 768)
                ps = psA.tile([128, 512], F32, tag="a")
                nc.tensor.matmul(ps[0:64, 0 : e - s], ones1f[:], cr_sb[:, s:e],
                                 start=True, stop=True)
                nc.vector.tensor_copy(consts[:, s:e], ps[0:64, 0 : e - s])
            CB_GPB = consts[:, 0:64]
            CB_GAOB = consts[:, 64:128]
            CB_FB1 = consts[:, 128:192]
            CB_FB2 = consts[:, 192:256]
            CB_N1W = consts[:, 256:320]
            CB_N1B = consts[:, 320:384]
            CB_N2W = consts[:, 384:448]
            CB_N2B = consts[:, 448:512]

            # s/r embedding rows for both batches
            srows = wp.tile([B2, HID], BF16, tag="srows")
            nc.gpsimd.indirect_dma_start(
                out=srows[:], out_offset=None, in_=sub_emb[:],
                in_offset=IndirectOffsetOnAxis(ap=sid_sb[:, 0:1], axis=0))
            rrows = wp.tile([B2, HID], BF16, tag="rrows")
            nc.gpsimd.indirect_dma_start(
                out=rrows[:], out_offset=None, in_=rel_emb[:],
                in_offset=IndirectOffsetOnAxis(ap=rid_sb[:, 0:1], axis=0))

            def ln_pair(x_f32, w_bc, b_bc, out_bf, tag):
                """LayerNorm over free axis of [64, 64] f32 -> bf16 out."""
                mu = rot.tile([64, 1], F32, tag=tag + "mu")
                nc.vector.reduce_sum(mu[:], x_f32[:], axis=AX.X)
                nc.vector.tensor_scalar_mul(mu[:], mu[:], 1.0 / GP)
                cent = rot.tile([64, GP], F32, tag=tag + "ct")
                nc.vector.tensor_scalar(cent[:], x_f32[:], mu[:, 0:1], None,
                                        op0=OP.subtract)
                sq = rot.tile([64, GP], F32, tag=tag + "sq")
                nc.vector.tensor_mul(sq[:], cent[:], cent[:])
                var = rot.tile([64, 1], F32, tag=tag + "vr")
                nc.vector.reduce_sum(var[:], sq[:], axis=AX.X)
                nc.vector.tensor_scalar(var[:], var[:], 1.0 / GP, 1e-5,
                                        op0=OP.mult, op1=OP.add)
                sd = rot.tile([64, 1], F32, tag=tag + "sd")
                nc.scalar.sqrt(sd[:], var[:])
                rstd = rot.tile([64, 1], F32, tag=tag + "rs")
                nc.vector.reciprocal(rstd[:], sd[:])
                nc.vector.tensor_scalar_mul(cent[:], cent[:], rstd[:, 0:1])
                of = rot.tile([64, GP], F32, tag=tag + "of")
                nc.vector.tensor_mul(of[:], cent[:], w_bc)
                nc.vector.tensor_add(of[:], of[:], b_bc)
                nc.vector.tensor_copy(out_bf[:], of[:])
                return of

            enhT = []
            for b in range(B2):
                sfx = str(b)
                # ---------------- per-batch loads ----------------
                def load128x4(dram, dtype, name):
                    t = wp.tile([128, 4], dtype, tag=name + sfx)
                    nc.sync.dma_start(
                        t[:], dram[b : b + 1, :].rearrange("o (c p) -> (o p) c", p=128))
                    return t

                objs = load128x4(objs_d, I32, "objs")
                subs = load128x4(subs_d, I32, "subs")
                marks = load128x4(marks_d, I32, "marks")
                fm = load128x4(fm_d, F32, "fm")
                th = wp.tile([1, L], F32, tag="th" + sfx)
                nc.sync.dma_start(th[:], th_d[b : b + 1, :])
                tq = wp.tile([1, L], F32, tag="tq" + sfx)
                nc.sync.dma_start(tq[:], tq_d[b : b + 1, :])
                dth = wp.tile([1, L], F32, tag="dth" + sfx)
                nc.sync.dma_start(dth[:], dth_d[b : b + 1, :])

                # ---------------- group ids + one-hot ----------------
                keys = wp.tile([128, 4], I32, tag="keys" + sfx)
                nc.vector.tensor_scalar(keys[:], subs[:], NR, None, op0=OP.mult)
                nc.vector.tensor_add(keys[:], keys[:], marks[:])
                gid = wp.tile([128, 4], I32, tag="gid" + sfx)
                for c in range(QC):
                    nc.gpsimd.indirect_dma_start(
                        out=gid[:, c : c + 1], out_offset=None, in_=gmap[:],
                        in_offset=IndirectOffsetOnAxis(ap=keys[:, c : c + 1], axis=0))
                gidf = wp.tile([128, 4], F32, tag="gidf" + sfx)
                nc.vector.tensor_copy(gidf[:], gid[:])
                H = wp.tile([128, 4, NG], BF16, tag="H" + sfx)
                HT = wp.tile([64, 4, 128], BF16, tag="HT" + sfx)
                for c in range(QC):
                    nc.vector.tensor_tensor(
                        out=H[:, c, :], in0=gidf[:, c : c + 1].to_broadcast([128, NG]),
                        in1=iota_g[:], op=OP.is_equal)
                    nc.vector.tensor_scalar_mul(H[:, c, :], H[:, c, :], fm[:, c : c + 1])
                    pst = psA.tile([128, 512], BF16, tag="a")
                    nc.tensor.transpose(pst[0:64, 0:128], H[:, c, :], ident_b[:])
                    nc.vector.tensor_copy(HT[:, c, :], pst[0:64, 0:128])

                # ---------------- embeddings + time encodings ----------------
                xg = wp.tile([128, 4, DM], BF16, tag="xg" + sfx)
                for c in range(QC):
                    nc.gpsimd.indirect_dma_start(
                        out=xg[:, c, :], out_offset=None, in_=obj_emb[:],
                        in_offset=IndirectOffsetOnAxis(ap=objs[:, c : c + 1], axis=0))
                x_std = wp.tile([128, 4, DM], BF16, tag="xstd" + sfx)
                cur0_std = wp.tile([128, 4, DM], BF16, tag="c0std" + sfx)
                for c in range(QC):
                    sl = slice(c * 128, (c + 1) * 128)
                    pa = psA.tile([128, 512], F32, tag="a")
                    nc.tensor.matmul(pa[:, 0:128], th[:, sl], freqs[:],
                                     start=True, stop=True)
                    pb = psA.tile([128, 512], F32, tag="a")
                    nc.tensor.matmul(pb[:, 0:128], dth[:, sl], freqs[:],
                                     start=True, stop=True)
                    t1 = rot.tile([128, DM], F32, tag="te1")
                    nc.scalar.activation(t1[:, 0:128], pa[:, 0:128], AF.Sin)
                    nc.scalar.activation(t1[:, 128:256], pa[:, 0:128], AF.Sin, bias=pi2[:, 0:1])
                    t2 = rot.tile([128, DM], F32, tag="te2")
                    nc.scalar.activation(t2[:, 0:128], pb[:, 0:128], AF.Sin)
                    nc.scalar.activation(t2[:, 128:256], pb[:, 0:128], AF.Sin, bias=pi2[:, 0:1])
                    nc.vector.tensor_add(x_std[:, c, :], t1[:], t2[:])
                    nc.vector.tensor_add(x_std[:, c, :], x_std[:, c, :], xg[:, c, :])
                    pq = psA.tile([128, 512], F32, tag="a")
                    nc.tensor.matmul(pq[:, 0:128], tq[:, sl], freqs[:],
                                     start=True, stop=True)
                    nc.scalar.activation(cur0_std[:, c, 0:128], pq[:, 0:128], AF.Sin)
                    nc.scalar.activation(cur0_std[:, c, 128:256], pq[:, 0:128],
                                         AF.Sin, bias=pi2[:, 0:1])

                # transposes into feature-major
                xT = wp.tile([128, 2, L], BF16, tag="xT" + sfx)
                cur0T = wp.tile([128, 2, L], BF16, tag="c0T" + sfx)
                for c in range(QC):
                    for d in range(2):
                        pt = psA.tile([128, 512], BF16, tag="a")
                        nc.tensor.transpose(
                            pt[:, 0:128], x_std[:, c, d * 128 : (d + 1) * 128], ident_b[:])
                        nc.vector.tensor_copy(
                            xT[:, d, c * 128 : (c + 1) * 128], pt[:, 0:128])
                        pt2 = psA.tile([128, 512], BF16, tag="a")
                        nc.tensor.transpose(
                            pt2[:, 0:128], cur0_std[:, c, d * 128 : (d + 1) * 128],
                            ident_b[:])
                        nc.vector.tensor_copy(
                            cur0T[:, d, c * 128 : (c + 1) * 128], pt2[:, 0:128])

                mergedT = wp.tile([128, DK, L], BF16, tag="mergedT" + sfx)
                feats = wp.tile([128, 4, DF + 1], BF16, tag="feats" + sfx)
                srow = wp.tile([1, HID], BF16, tag="srow" + sfx)
                nc.sync.dma_start(srow[:], srows[b : b + 1, :])
                rrow = wp.tile([1, HID], BF16, tag="rrow" + sfx)
                nc.sync.dma_start(rrow[:], rrows[b : b + 1, :])

                # ---------------- attention layers ----------------
                for l in range(NL):
                    curT = cur0T[:, :, :] if l == 0 else mergedT[:, 0:2, :]
                    qT = rot.tile([128, 2, L], BF16, tag="qT")
                    kT = rot.tile([128, 2, L], BF16, tag="kT")
                    for t_out, w_in, src in ((qT, wq, curT), (kT, wk, xT[:, :, :])):
                        for mc in range(2):
                            ms = slice(mc * 128, (mc + 1) * 128)
                            ps = psA.tile([128, 512], F32, tag="a")
                            for dc in range(2):
                                nc.tensor.matmul(ps[:], w_in[:, l, dc, ms], src[:, dc, :],
                                                 start=(dc == 0), stop=(dc == 1))
                            nc.vector.tensor_copy(t_out[:, mc, :], ps[:])
                    v_ext = rot.tile([128, 4, NH, HD + 1], BF16, tag="vext")
                    nc.vector.memset(v_ext[:, :, :, HD : HD + 1], 1.0)
                    for kc in range(QC):
                        ks = slice(kc * 128, (kc + 1) * 128)
                        ps = psA.tile([128, 512], F32, tag="a")
                        for dc in range(2):
                            nc.tensor.matmul(ps[:, 0:DM], xT[:, dc, ks], wv[:, l, dc, :],
                                             start=(dc == 0), stop=(dc == 1))
                        nc.vector.tensor_copy(
                            v_ext[:, kc, :, 0:HD],
                            ps[:, 0:DM].rearrange("p (h d) -> p h d", h=NH))
                    attnT = rot.tile([128, 2, L], BF16, tag="attnT")
                    for hp in range(2):
                        avs = []
                        for hl in range(2):
                            h = 2 * hp + hl
                            ro = slice(64 * hl, 64 * hl + 64)
                            av = psA.tile([128, 512], F32, tag="a")
                            for kc in range(QC):
                                n0 = 128 * kc
                                nq = L - n0
                                sp = psA.tile([128, 512], F32, tag="a")
                                nc.tensor.matmul(
                                    sp[:, 0:nq], kT[ro, hp, n0 : n0 + 128],
                                    qT[ro, hp, n0:L], start=True, stop=True)
                                me = rot.tile([128, L], BF16, tag="me")
                                nc.scalar.activation(me[:, 0:nq], sp[:, 0:nq],
                                                     AF.Exp, scale=0.125)
                                nc.vector.tensor_mul(me[:, 0:128], me[:, 0:128], tri[:])
                                nc.tensor.matmul(
                                    av[0:65, n0:L], v_ext[:, kc, h, :], me[:, 0:nq],
                                    start=(kc == 0), stop=(kc == 3))
                            avs.append(av)
                        rsel0 = rot.tile([1, L], F32, tag="rsel0")
                        nc.vector.reciprocal(rsel0[:], avs[0][64:65, :])
                        rsel1 = rot.tile([1, L], F32, tag="rsel1")
                        nc.vector.reciprocal(rsel1[:], avs[1][64:65, :])
                        mult = psA.tile([128, 512], F32, tag="a")
                        nc.tensor.matmul(mult[:], selA[:], rsel0[:], start=True, stop=False)
                        nc.tensor.matmul(mult[:], selB[:], rsel1[:], start=False, stop=True)
                        avsb = rot.tile([128, L], BF16, tag="avsb")
                        nc.vector.tensor_copy(avsb[0:64, :], avs[0][0:64, :])
                        nc.vector.tensor_copy(avsb[64:128, :], avs[1][0:64, :])
                        nc.vector.tensor_mul(attnT[:, hp, :], avsb[:], mult[:])
                    # output proj + residuals, both layouts
                    for mc in range(2):
                        ms = slice(mc * 128, (mc + 1) * 128)
                        ps = psA.tile([128, 512], F32, tag="a")
                        for dc in range(2):
                            nc.tensor.matmul(ps[:], wo[:, l, dc, ms], attnT[:, dc, :],
                                             start=(dc == 0), stop=(dc == 1))
                        nc.vector.tensor_add(mergedT[:, 2 * l + mc, :], curT[:, mc, :],
                                             ps[:])
                    for qc in range(QC):
                        qs = slice(qc * 128, (qc + 1) * 128)
                        ps = psA.tile([128, 512], F32, tag="a")
                        for dc in range(2):
                            nc.tensor.matmul(ps[:, 0:DM], attnT[:, dc, qs], wo[:, l, dc, :],
                                             start=(dc == 0), stop=(dc == 1))
                        if l == 0:
                            nc.vector.tensor_add(feats[:, qc, 0:DM], cur0_std[:, qc, :],
                                                 ps[:, 0:DM])
                        else:
                            nc.vector.tensor_add(feats[:, qc, DM : 2 * DM],
                                                 feats[:, qc, 0:DM], ps[:, 0:DM])

                # ---------------- s/r embedding broadcasts ----------------
                for base_c, row in ((4, srow), (6, rrow)):
                    for dc in range(2):
                        ps = psA.tile([128, 512], F32, tag="a")
                        nc.tensor.matmul(ps[:], row[:, dc * 128 : (dc + 1) * 128],
                                         ones_row[:], start=True, stop=True)
                        nc.vector.tensor_copy(mergedT[:, base_c + dc, :], ps[:])
                for qc in range(QC):
                    ps = psA.tile([128, 512], F32, tag="a")
                    nc.tensor.matmul(ps[:, 0:HID], ones_row[:, 0:128], srow[:],
                                     start=True, stop=True)
                    nc.vector.tensor_copy(feats[:, qc, 2 * DM : 2 * DM + HID],
                                          ps[:, 0:HID])
                    ps2 = psA.tile([128, 512], F32, tag="a")
                    nc.tensor.matmul(ps2[:, 0:HID], ones_row[:, 0:128], rrow[:],
                                     start=True, stop=True)
                    nc.vector.tensor_copy(feats[:, qc, 2 * DM + HID : DF], ps2[:, 0:HID])
                nc.vector.memset(feats[:, :, DF : DF + 1], 1.0)

                # ---------------- segment mean (scatter via one-hot matmul) ----------
                cnt_ps = psA.tile([128, 512], F32, tag="a")
                for qc in range(QC):
                    nc.tensor.matmul(cnt_ps[0:NG, 0:1], H[:, qc, :],
                                     feats[:, qc, DF : DF + 1],
                                     start=(qc == 0), stop=(qc == 3))
                cnt = rot.tile([NG, 1], F32, tag="cnt")
                nc.vector.tensor_scalar_max(cnt[:], cnt_ps[0:NG, 0:1], 1.0)
                recip = rot.tile([NG, 1], F32, tag="recip")
                nc.vector.reciprocal(recip[:], cnt[:])
                rep = rot.tile([NG, DF], BF16, tag="rep")
                for half in range(2):
                    hs = slice(half * 512, (half + 1) * 512)
                    gs = psA.tile([128, 512], F32, tag="a")
                    for qc in range(QC):
                        nc.tensor.matmul(gs[0:NG, :], H[:, qc, :], feats[:, qc, hs],
                                         start=(qc == 0), stop=(qc == 3))
                    nc.vector.tensor_scalar_mul(rep[:, hs], gs[0:NG, :], recip[:, 0:1])
                repT = rot.tile([128, 8, NG], BF16, tag="repT")
                for c in range(8):
                    pt = psA.tile([128, 512], BF16, tag="a")
                    nc.tensor.transpose(pt[:, 0:NG], rep[:, c * 128 : (c + 1) * 128],
                                        ident_b[0:NG, 0:NG])
                    nc.vector.tensor_copy(repT[:, c, :], pt[:, 0:NG])

                # ---------------- group transformer ----------------
                gp_ps = psA.tile([128, 512], F32, tag="a")
                for c in range(8):
                    nc.tensor.matmul(gp_ps[0:GP, 0:GP], repT[:, c, :], gpw[:, c, :],
                                     start=(c == 0), stop=(c == 7))
                gp_f = rot.tile([GP, GP], F32, tag="gpf")
                nc.vector.tensor_add(gp_f[:], gp_ps[0:GP, 0:GP], CB_GPB)
                gp_bf = rot.tile([GP, GP], BF16, tag="gpbf")
                nc.vector.tensor_copy(gp_bf[:], gp_f[:])
                pt = psA.tile([128, 512], BF16, tag="a")
                nc.tensor.transpose(pt[0:GP, 0:GP], gp_bf[:], ident_b[0:GP, 0:GP])
                gpT = rot.tile([GP, GP], BF16, tag="gpT")
                nc.vector.tensor_copy(gpT[:], pt[0:GP, 0:GP])
                qk_ps = psA.tile([128, 512], F32, tag="a")
                nc.tensor.matmul(qk_ps[:, 0:GP], gaiw[:, 0:128], gpT[:],
                                 start=True, stop=True)
                qTg = rot.tile([GP, GP], BF16, tag="qTg")
                nc.vector.tensor_scalar(qTg[:], qk_ps[0:64, 0:GP], gab1[0:64, 0:1],
                                        None, op0=OP.add)
                kTg = rot.tile([GP, GP], BF16, tag="kTg")
                nc.vector.tensor_scalar(kTg[:], qk_ps[64:128, 0:GP], gab1[64:128, 0:1],
                                        None, op0=OP.add)
                v_ps = psA.tile([128, 512], F32, tag="a")
                nc.tensor.matmul(v_ps[0:GP, 0:GP], gaiw[:, 128:192], gpT[:],
                                 start=True, stop=True)
                vT = rot.tile([GP, GP], BF16, tag="vT")
                nc.vector.tensor_scalar(vT[:], v_ps[0:GP, 0:GP], gab2[:, 0:1], None,
                                        op0=OP.add)
                pv = psA.tile([128, 512], BF16, tag="a")
                nc.tensor.transpose(pv[0:GP, 0:GP], vT[:], ident_b[0:GP, 0:GP])
                vg_ext = rot.tile([GP, 2, 33], BF16, tag="vgext")
                nc.vector.memset(vg_ext[:, :, 32:33], 1.0)
                nc.vector.tensor_copy(
                    vg_ext[:, :, 0:32],
                    pv[0:GP, 0:GP].rearrange("p (h d) -> p h d", h=2))
                attn_g = rot.tile([GP, GP], BF16, tag="attng")
                for h in range(2):
                    sg = psA.tile([128, 512], F32, tag="a")
                    nc.tensor.matmul(sg[0:GP, 0:GP], kTg[32 * h : 32 * h + 32, :],
                                     qTg[32 * h : 32 * h + 32, :], start=True, stop=True)
                    eg = rot.tile([GP, GP], BF16, tag="eg")
                    nc.scalar.activation(eg[:], sg[0:GP, 0:GP], AF.Exp,
                                         scale=float(1.0 / np.sqrt(32.0)))
                    ag = psA.tile([128, 512], F32, tag="a")
                    nc.tensor.matmul(ag[0:33, 0:GP], vg_ext[:, h, :], eg[:],
                                     start=True, stop=True)
                    ag_sb = rot.tile([33, GP], BF16, tag="agsb")
                    nc.vector.tensor_copy(ag_sb[:], ag[0:33, 0:GP])
                    agT = psA.tile([128, 512], BF16, tag="a")
                    nc.tensor.transpose(agT[0:GP, 0:33], ag_sb[:], ident_b[0:33, 0:33])
                    rec_g = rot.tile([GP, 1], F32, tag="recg")
                    nc.vector.reciprocal(rec_g[:], agT[0:GP, 32:33])
                    nc.vector.tensor_scalar_mul(attn_g[:, 32 * h : 32 * h + 32],
                                                agT[0:GP, 0:32], rec_g[:, 0:1])
                pat = psA.tile([128, 512], BF16, tag="a")
                nc.tensor.transpose(pat[0:GP, 0:GP], attn_g[:], ident_b[0:GP, 0:GP])
                attn_gT = rot.tile([GP, GP], BF16, tag="attngT")
                nc.vector.tensor_copy(attn_gT[:], pat[0:GP, 0:GP])
                ga_ps = psA.tile([128, 512], F32, tag="a")
                nc.tensor.matmul(ga_ps[0:GP, 0:GP], attn_gT[:], gaow[:],
                                 start=True, stop=True)
                gn_in = rot.tile([GP, GP], F32, tag="gnin")
                nc.vector.tensor_add(gn_in[:], ga_ps[0:GP, 0:GP], CB_GAOB)
                nc.vector.tensor_add(gn_in[:], gn_in[:], gp_f[:])
                gn_bf = rot.tile([GP, GP], BF16, tag="gnbf")
                gn_f = ln_pair(gn_in, CB_N1W, CB_N1B, gn_bf, "l1")
                pgn = psA.tile([128, 512], BF16, tag="a")
                nc.tensor.transpose(pgn[0:GP, 0:GP], gn_bf[:], ident_b[0:GP, 0:GP])
                gnT = rot.tile([GP, GP], BF16, tag="gnT")
                nc.vector.tensor_copy(gnT[:], pgn[0:GP, 0:GP])
                f1_ps = psA.tile([128, 512], F32, tag="a")
                nc.tensor.matmul(f1_ps[0:GP, 0:GP], gnT[:], ffn1[:], start=True, stop=True)
                f1f = rot.tile([GP, GP], F32, tag="f1f")
                nc.vector.tensor_add(f1f[:], f1_ps[0:GP, 0:GP], CB_FB1)
                f1b = rot.tile([GP, GP], BF16, tag="f1b")
                nc.vector.tensor_relu(f1b[:], f1f[:])
                pf1 = psA.tile([128, 512], BF16, tag="a")
                nc.tensor.transpose(pf1[0:GP, 0:GP], f1b[:], ident_b[0:GP, 0:GP])
                f1T = rot.tile([GP, GP], BF16, tag="f1T")
                nc.vector.tensor_copy(f1T[:], pf1[0:GP, 0:GP])
                f2_ps = psA.tile([128, 512], F32, tag="a")
                nc.tensor.matmul(f2_ps[0:GP, 0:GP], f1T[:], ffn2[:], start=True, stop=True)
                go_in = rot.tile([GP, GP], F32, tag="goin")
                nc.vector.tensor_add(go_in[:], f2_ps[0:GP, 0:GP], CB_FB2)
                nc.vector.tensor_add(go_in[:], go_in[:], gn_f[:])
                gout = rot.tile([GP, GP], BF16, tag="gout")
                ln_pair(go_in, CB_N2W, CB_N2B, gout, "l2")

                # gatheredT into mergedT chunk 8
                gath = psA.tile([128, 512], F32, tag="a")
                nc.tensor.matmul(gath[0:GP, :], gout[:],
                                 HT[:, :, :].rearrange("p c q -> p (c q)"),
                                 start=True, stop=True)
                nc.vector.tensor_copy(mergedT[0:64, 8, :], gath[0:GP, :])
                nc.vector.memset(mergedT[64:128, 8, :], 0.0)
                nc.vector.memset(mergedT[64:65, 8, :], 1.0)

                # ---------------- merge linear -> enhancedT ----------------
                eT = wp.tile([128, DK, L], BF16, tag="enhT" + sfx)
                for mc in range(8):
                    ms = slice(mc * 128, (mc + 1) * 128)
                    mp = psA.tile([128, 512], F32, tag="a")
                    for kc in range(8):
                        nc.tensor.matmul(mp[:], mgw[:, kc, ms], mergedT[:, kc, :],
                                         start=(kc == 0), stop=False)
                    nc.tensor.matmul(mp[:], mgw[0:65, 8, ms], mergedT[0:65, 8, :],
                                     start=False, stop=True)
                    nc.vector.tensor_copy(eT[:, mc, :], mp[:])
                nc.vector.memset(eT[:, 8, :], 0.0)
                nc.vector.memset(eT[0:1, 8, :], 1.0)
                enhT.append(eT)

            # ---------------- intensity head ----------------
            for nt in range(NT):
                ns = slice(nt * NTW, (nt + 1) * NTW)
                w = iwp.tile([128, DK, NTW], BF16, tag="intw")
                nc.sync.dma_start(w[:, 0:8, :], intw_d[:, 0:8, ns])
                nc.sync.dma_start(w[0:1, 8, :], intw_d[0:1, 8, ns])
                for b in range(B2):
                    for qc in range(QC):
                        qs = slice(qc * 128, (qc + 1) * 128)
                        acc = psI.tile([128, NTW], F32, tag="i")
                        for kc in range(8):
                            nc.tensor.matmul(acc[:], enhT[b][:, kc, qs], w[:, kc, :],
                                             start=(kc == 0), stop=False)
                        nc.tensor.matmul(acc[:], enhT[b][0:1, 8, qs], w[0:1, 8, :],
                                         start=False, stop=True)
                        o = ost.tile([128, NTW], F32, tag="o")
                        nc.scalar.activation(o[:], acc[:], AF.Exp)
                        nc.vector.tensor_scalar_add(o[:], o[:], 1.0)
                        nc.scalar.activation(o[:], o[:], AF.Ln)
                        nc.sync.dma_start(out_d[b, qs, ns], o[:])
    _split_excess_waits(nc)
    return nc


def _prep_shared(inp):
    bf = lambda a: np.asarray(a, np.float32).astype(ml_dtypes.bfloat16)
    f32 = lambda a: np.ascontiguousarray(np.asarray(a, np.float32))
    sh = {}
    sh["obj_emb"] = np.ascontiguousarray(bf(inp["obj_embed"]))
    sh["sub_emb"] = np.ascontiguousarray(bf(inp["sub_embed"]))
    sh["rel_emb"] = np.ascontiguousarray(bf(inp["rel_embed"]))
    sh["gmap"] = np.ascontiguousarray(
        np.asarray(inp["group_map"], np.int64).astype(np.int32).reshape(NE * NR, 1))
    for nm, key in (("wq", "core_Wq"), ("wk", "core_Wk"), ("wv", "core_Wv"),
                    ("wo", "core_Wo")):
        w = bf(inp[key]).reshape(NL, 2, 128, DM).transpose(2, 0, 1, 3)
        sh[nm] = np.ascontiguousarray(w)
    sh["gp_w"] = np.ascontiguousarray(
        bf(inp["gp_w"]).reshape(8, 128, GP).transpose(1, 0, 2))
    sh["ga_in_w"] = np.ascontiguousarray(bf(inp["ga_in_w"]))
    sh["ga_out_w"] = np.ascontiguousarray(bf(inp["ga_out_w"]))
    sh["ffn_w1"] = np.ascontiguousarray(bf(inp["ffn_w1"]))
    sh["ffn_w2"] = np.ascontiguousarray(bf(inp["ffn_w2"]))
    mg = np.zeros((1152, DF), np.float32)
    mg[0 : DF + GP] = np.asarray(inp["mg_w"], np.float32)
    mg[DF + GP] = np.asarray(inp["mg_b"], np.float32)
    sh["mg_w"] = np.ascontiguousarray(
        mg.astype(ml_dtypes.bfloat16).reshape(DK, 128, DF).transpose(1, 0, 2))
    iw = np.zeros((1152, NE), np.float32)
    iw[0:DF] = np.asarray(inp["int_w"], np.float32)
    iw[DF] = np.asarray(inp["int_b"], np.float32)
    sh["int_w"] = np.ascontiguousarray(
        iw.astype(ml_dtypes.bfloat16).reshape(DK, 128, NE).transpose(1, 0, 2))
    cr = np.zeros((1, 768), np.float32)
    for i, key in enumerate(("gp_b", "ga_out_b", "ffn_b1", "ffn_b2",
                             "n1_w", "n1_b", "n2_w", "n2_b")):
        cr[0, i * 64 : (i + 1) * 64] = np.asarray(inp[key], np.float32)
    sh["consts_row"] = cr
    sh["gab_col"] = f32(inp["ga_in_b"]).reshape(3 * GP, 1)
    i_ = np.arange(DM // 2)
    sh["freqs"] = np.exp(-np.log(10000.0) * (2.0 * i_ / DM)).astype(
        np.float32).reshape(1, 128)
    return sh


def _prep_core(inp, ci):
    bs = [B2 * ci + j for j in range(B2)]
    i32pad = lambda a: np.ascontiguousarray(
        np.pad(np.asarray(a, np.int64)[bs, :LH], ((0, 0), (0, 1))).astype(np.int32))
    f32pad = lambda a: np.ascontiguousarray(
        np.pad(np.asarray(a, np.float32)[bs, :LH], ((0, 0), (0, 1))))
    m = {}
    m["objs_h"] = i32pad(inp["objs"])
    m["subs_h"] = i32pad(inp["subs"])
    m["marks_h"] = i32pad(inp["marks"])
    m["sid"] = np.asarray(inp["subs"], np.int64)[bs, 0:1].astype(np.int32)
    m["rid"] = np.asarray(inp["marks"], np.int64)[bs, 0:1].astype(np.int32)
    m["th"] = f32pad(inp["times"])
    tqv = np.zeros((B2, L), np.float32)
    tqv[:, :LH] = np.asarray(inp["times"], np.float32)[bs, 1:]
    m["tq"] = tqv
    m["dth"] = f32pad(inp["dt"])
    m["fm"] = f32pad(np.asarray(inp["mask"], bool).astype(np.float32))
    return m


last_results = None


def kernel(**inputs) -> np.ndarray:
    global _cached, last_results
    if _cached is None:
        _cached = _build()
    nc = _cached
    sh = _prep_shared(inputs)
    in_maps = []
    for ci in range(NCORE):
        m = dict(sh)
        m.update(_prep_core(inputs, ci))
        in_maps.append(m)
    import os
    trace = bool(os.environ.get("BASS_TRACE"))
    res = run_bass_kernel_spmd(nc, in_maps, core_ids=list(range(NCORE)), trace=trace)
    last_results = res
    out = np.empty((B, LH, NE), np.float32)
    for ci in range(NCORE):
        o = res.results[ci]["out"]
        for j in range(B2):
            out[B2 * ci + j] = o[j, :LH, :]
    return out
